# revision 1
# baseline (speedup 1.0000x reference)
"""GroupedQueryAttention Trainium2 kernel (8 NeuronCores).

Sharding: (batch b in 0..1) x (kv-head group g in 0..3) -> core 4*b+g.
Each core computes, for its batch, the 4 query heads (4g..4g+3) that share
kv head g, plus the partial output projection through the matching 512-row
slice of Wo.  The host sums the 4 partials per batch.

On-device dataflow is fully "transposed": activations live as [feature,
token] so every matmul contraction sits on the partition axis, and the
softmax probabilities come out directly in the layout the P@V matmul
needs (no on-chip transposes of the attention matrix).  Softmax
denominators come from an all-ones stationary matmul over the same
probability tiles, which also yields them pre-broadcast across partitions.
Causality is exploited by only computing score tiles on/below the block
diagonal; the block-diagonal tiles are masked with a 0/1 step+triangle
pattern after the exp.
"""

import numpy as np
import ml_dtypes

DIM, H, KV, S, B = 2048, 16, 4, 2048, 2
HD = DIM // H          # 128
GQ = H // KV           # 4 query heads per kv head
P = 128                # partitions
NK = DIM // P          # 16 contraction tiles
NCH = S // 512         # 4 sequence chunks of 512
EPS = 1e-6
BF = ml_dtypes.bfloat16

_CACHED = {}


def _build_program():
    import concourse.bass as bass
    import concourse.tile as tile
    from concourse import bacc
    from concourse import mybir
    from concourse.masks import make_identity

    f32 = mybir.dt.float32
    bf16 = mybir.dt.bfloat16
    AF = mybir.ActivationFunctionType

    nc = bacc.Bacc()
    xT = nc.declare_dram_parameter("xT", [DIM, S], bf16, isOutput=False)
    wq = nc.declare_dram_parameter("wq", [DIM, GQ * HD], bf16, isOutput=False)
    wk = nc.declare_dram_parameter("wk", [DIM, HD], bf16, isOutput=False)
    wv = nc.declare_dram_parameter("wv", [DIM, HD], bf16, isOutput=False)
    wo = nc.declare_dram_parameter("wo", [GQ * HD, DIM], bf16, isOutput=False)
    cosq = nc.declare_dram_parameter("cosq", [HD, S], bf16, isOutput=False)
    sinq = nc.declare_dram_parameter("sinq", [HD, S], bf16, isOutput=False)
    cosk = nc.declare_dram_parameter("cosk", [HD, S], bf16, isOutput=False)
    sink = nc.declare_dram_parameter("sink", [HD, S], bf16, isOutput=False)
    m4 = nc.declare_dram_parameter("m4", [4, P, 512], bf16, isOutput=False)
    rsw = nc.declare_dram_parameter("rsw", [P, P], bf16, isOutput=False)
    po = nc.declare_dram_parameter("po", [S, DIM], f32, isOutput=True)

    inv_sqrt_hd = 1.0 / float(np.sqrt(HD))

    with tile.TileContext(nc) as tc:
      with tc.tile_pool(name="const", bufs=1) as const, \
           tc.tile_pool(name="hatp", bufs=1) as hatp, \
           tc.tile_pool(name="w5", bufs=2) as w5, \
           tc.tile_pool(name="m4p", bufs=1) as m4p, \
           tc.tile_pool(name="csp", bufs=1) as csp:
        ones_sb = const.tile([P, P], bf16)
        nc.vector.memset(ones_sb, 1.0)
        ident = const.tile([P, P], bf16)
        make_identity(nc, ident)
        epsb = const.tile([P, 1], f32)
        nc.vector.memset(epsb, EPS)
        rsw_sb = const.tile([P, P], bf16)
        nc.scalar.dma_start(out=rsw_sb, in_=rsw[:, :])

        # prefetched during P1 (emitted after chunk 0 so they queue behind it)
        wo_sb = w5.tile([P, GQ, DIM], bf16, bufs=1)
        m4_sb = m4p.tile([P, 4, 512], bf16)
        cs_sb = {}
        for nm in ("cosq", "sinq", "cosk", "sink"):
            cs_sb[nm] = csp.tile([P, S], bf16, tag=f"cs_{nm}", name=f"cs_{nm}")

        v_nat = hatp.tile([P, NK, HD], bf16, tag="vnat")
        onorm = [hatp.tile([P, S], bf16, tag=f"onorm{h}", name=f"onorm{h}")
                 for h in range(GQ)]
        qhat = [hatp.tile([P, S], bf16, tag=f"qhat{h}", name=f"qhat{h}")
                for h in range(GQ)]
        khat = hatp.tile([P, S], bf16, tag="khat")

        with tc.tile_pool(name="qkvp", bufs=1) as qkvp:
            q32 = [qkvp.tile([P, S], bf16, tag=f"qp_{h}", name=f"qp_{h}")
                   for h in range(GQ)]
            k32 = qkvp.tile([P, S], bf16, tag="kp")
            vT_bf = qkvp.tile([P, S], bf16, tag="vT")

            # ---- P1: projections (transposed outputs) + v transpose ----
            with tc.tile_pool(name="xt", bufs=1) as xtp, \
                 tc.tile_pool(name="p1ps", bufs=3, space="PSUM") as p1ps:
                wk_sb = xtp.tile([P, NK, HD], bf16)
                nc.sync.dma_start(out=wk_sb, in_=wk.ap().rearrange("(j p) n -> p j n", p=P))
                xt_sb = xtp.tile([P, NK, S], bf16)
                xt_src = xT.ap().rearrange("(j p) t -> p j t", p=P)
                nc.sync.dma_start(out=xt_sb[:, :, 0:512], in_=xt_src[:, :, 0:512])
                wq_sb = xtp.tile([P, NK, GQ * HD], bf16)
                nc.sync.dma_start(out=wq_sb, in_=wq.ap().rearrange("(j p) n -> p j n", p=P))
                wv_sb = xtp.tile([P, NK, HD], bf16)
                nc.sync.dma_start(out=wv_sb, in_=wv.ap().rearrange("(j p) n -> p j n", p=P))
                for c in range(1, NCH):
                    csl = slice(c * 512, (c + 1) * 512)
                    nc.sync.dma_start(out=xt_sb[:, :, csl], in_=xt_src[:, :, csl])
                # k first: khat is needed by every score tile in P3
                for c in range(NCH):
                    sl = slice(c * 512, (c + 1) * 512)
                    for slot in (4, 0, 1, 2, 3, 5):
                        ps = p1ps.tile([P, 512], f32, tag="proj")
                        for j in range(NK):
                            if slot < 4:
                                lhs = wq_sb[:, j, slot * HD:(slot + 1) * HD]
                            elif slot == 4:
                                lhs = wk_sb[:, j, :]
                            else:
                                lhs = wv_sb[:, j, :]
                            nc.tensor.matmul(ps, lhs, xt_sb[:, j, sl],
                                             start=(j == 0), stop=(j == NK - 1))
                        if slot < 4:
                            nc.scalar.copy(q32[slot][:, sl], ps)
                        elif slot == 4:
                            nc.scalar.copy(k32[:, sl], ps)
                        else:
                            nc.scalar.copy(vT_bf[:, sl], ps)
                nc.scalar.dma_start(out=wo_sb, in_=wo.ap().rearrange("(h p) n -> p h n", p=P))
                nc.scalar.dma_start(out=m4_sb, in_=m4.ap().rearrange("a p n -> p a n"))
                for nm, t in (("cosq", cosq), ("sinq", sinq), ("cosk", cosk), ("sink", sink)):
                    nc.scalar.dma_start(out=cs_sb[nm], in_=t[:, :])
                # v natural layout [sk_local, j, d] via PE transpose of vT
                for j in range(NK):
                    tp = p1ps.tile([P, HD], bf16, tag="vtr")
                    nc.tensor.transpose(tp, vT_bf[:, j * HD:(j + 1) * HD], ident)
                    nc.scalar.copy(v_nat[:, j, :], tp)

            # ---- P2: rmsnorm (pre-gain) + rope, full-row ops, k first ----
            with tc.tile_pool(name="w2", bufs=2) as w2, \
                 tc.tile_pool(name="p2ps", bufs=2, space="PSUM") as p2ps:
                for t in (4, 0, 1, 2, 3):
                    src = q32[t] if t < 4 else k32
                    dst = qhat[t] if t < 4 else khat
                    cosT = cs_sb["cosq" if t < 4 else "cosk"]
                    sinT = cs_sb["sinq" if t < 4 else "sink"]
                    # sum of squares over feature (partition) axis via
                    # all-ones matmul; arrives replicated on all partitions
                    sqb = w2.tile([P, S], bf16, tag="sqb")
                    nc.scalar.activation(sqb, src, AF.Square)
                    ssq = p2ps.tile([P, S], f32, tag="ssq", bufs=1)
                    rot = p2ps.tile([P, S // 2], f32, tag="rot", bufs=2)
                    rot2 = p2ps.tile([P, S // 2], f32, tag="rot", bufs=2)
                    for c in range(NCH):
                        sl = slice(c * 512, (c + 1) * 512)
                        nc.tensor.matmul(ssq[:, sl], ones_sb, sqb[:, sl],
                                         start=True, stop=True)
                        rt = rot if c < 2 else rot2
                        rsl = slice((c % 2) * 512, (c % 2 + 1) * 512)
                        nc.tensor.matmul(rt[:, rsl], rsw_sb, src[:, sl],
                                         start=True, stop=True)
                    lnb = w2.tile([P, S], f32, tag="lnb")
                    nc.scalar.activation(lnb, ssq, AF.Ln, bias=epsb, scale=1.0 / HD)
                    rsb = w2.tile([P, S], bf16, tag="rsb")
                    nc.scalar.activation(rsb, lnb, AF.Exp, scale=-0.5)
                    # rope: y = src*cos + rot(src)*sin (sign/gain in tables)
                    t1 = w2.tile([P, S], bf16, tag="t1")
                    nc.vector.tensor_mul(t1, src, cosT)
                    t2 = w2.tile([P, S], bf16, tag="t2")
                    nc.vector.tensor_mul(t2[:, 0:1024], rot, sinT[:, 0:1024])
                    nc.vector.tensor_mul(t2[:, 1024:2048], rot2, sinT[:, 1024:2048])
                    t3 = w2.tile([P, S], bf16, tag="t3")
                    nc.vector.tensor_add(t3, t1, t2)
                    nc.vector.tensor_mul(dst, t3, rsb)

        # ---- P3: attention, all heads per chunk ----
        with tc.tile_pool(name="wep", bufs=2) as wep:
          with tc.tile_pool(name="ptp", bufs=36) as ptp, \
               tc.tile_pool(name="p3s", bufs=2, space="PSUM") as p3s, \
               tc.tile_pool(name="p3o", bufs=4, space="PSUM") as p3o:
              for c in range(NCH):
                  sl = slice(c * 512, (c + 1) * 512)
                  nj = 4 * c + 4
                  # scores + exp, two 512-wide sk-tiles per PSUM tile so the
                  # exp runs 1024 wide
                  ptsc = {}
                  for h in range(GQ):
                      for pr in range(nj // 2):
                          sc = p3s.tile([P, 1024], f32, tag="sc",
                                        name=f"sc_{c}_{h}_{pr}")
                          for u in range(2):
                              j = 2 * pr + u
                              nc.tensor.matmul(sc[:, u * 512:(u + 1) * 512],
                                               khat[:, j * P:(j + 1) * P],
                                               qhat[h][:, sl],
                                               start=True, stop=True)
                          pt = ptp.tile([P, 1024], bf16, tag="pt",
                                        name=f"pt_{c}_{h}_{pr}")
                          nc.scalar.activation(pt, sc, AF.Exp, scale=inv_sqrt_hd)
                          for u in range(2):
                              j = 2 * pr + u
                              if j >= 4 * c:
                                  usl = slice(u * 512, (u + 1) * 512)
                                  nc.vector.tensor_mul(pt[:, usl], pt[:, usl],
                                                   m4_sb[:, j - 4 * c, :])
                          ptsc[(h, pr)] = pt
                  # P@V, j-outer so the stationary v tile is reused across heads
                  ots = [p3o.tile([P, 512], f32, tag="ot", name=f"ot_{c}_{h}")
                         for h in range(GQ)]
                  for j in range(nj):
                      usl = slice((j % 2) * 512, (j % 2 + 1) * 512)
                      for h in range(GQ):
                          nc.tensor.matmul(ots[h], v_nat[:, j, :],
                                           ptsc[(h, j // 2)][:, usl],
                                           start=(j == 0), stop=(j == nj - 1))
                  # denominators (replicated across partitions by the all-ones
                  # stationary; den tiles reuse the sc slots), then normalize
                  for h in range(GQ):
                      den = p3s.tile([P, 512], f32, tag="sc", name=f"den_{c}_{h}")
                      for j in range(nj):
                          usl = slice((j % 2) * 512, (j % 2 + 1) * 512)
                          nc.tensor.matmul(den, ones_sb,
                                           ptsc[(h, j // 2)][:, usl],
                                           start=(j == 0), stop=(j == nj - 1))
                      rec = wep.tile([P, 512], f32, tag="rec")
                      nc.vector.reciprocal(rec, den)
                      nc.vector.tensor_mul(onorm[h][:, sl], ots[h], rec)

          # ---- P5: partial output projection: po = onorm^T @ Wo_g ----
          with tc.tile_pool(name="p5ps", bufs=5, space="PSUM") as p5ps:
              for i in range(S // P):
                  isl = slice(i * P, (i + 1) * P)
                  po_ps = [p5ps.tile([P, 512], f32, tag="po", name=f"po_{i}_{n2}")
                           for n2 in range(NCH)]
                  for h in range(GQ):
                      for n in range(NCH):
                          nc.tensor.matmul(po_ps[n], onorm[h][:, isl],
                                           wo_sb[:, h, n * 512:(n + 1) * 512],
                                           start=(h == 0), stop=(h == GQ - 1))
                  row = wep.tile([P, DIM], f32, tag="row")
                  for n in range(NCH):
                      if n % 2 == 0:
                          nc.scalar.copy(row[:, n * 512:(n + 1) * 512], po_ps[n])
                      else:
                          nc.vector.tensor_copy(row[:, n * 512:(n + 1) * 512], po_ps[n])
                  nc.sync.dma_start(out=po[isl, :], in_=row)
    nc.compile()
    return nc


def _causal_ok(mask):
    m = np.asarray(mask).reshape(S, S)
    tri = np.tril(np.ones((S, S), dtype=bool))
    return bool(np.all(m[tri] == 0.0) and np.all(m[~tri] <= -1e8))


def _reference_fallback(x, Wq, Wk, Wv, Wo, qg, kg, cos, sin, mask):
    x64 = np.asarray(x, dtype=np.float32)
    q = (x64 @ Wq).reshape(B, S, H, HD).transpose(0, 2, 1, 3)
    k = (x64 @ Wk).reshape(B, S, KV, HD).transpose(0, 2, 1, 3)
    v = (x64 @ Wv).reshape(B, S, KV, HD).transpose(0, 2, 1, 3)

    def rms(t, g):
        r = np.sqrt(np.mean(t * t, axis=-1, keepdims=True) + EPS)
        return g * (t / r)

    q, k = rms(q, qg), rms(k, kg)

    def rot(t):
        return np.concatenate([-t[..., HD // 2:], t[..., :HD // 2]], axis=-1)

    c = cos[None, None, :, :]
    s = sin[None, None, :, :]
    q = q * c + rot(q) * s
    k = k * c + rot(k) * s
    k = np.repeat(k, GQ, axis=1)
    v = np.repeat(v, GQ, axis=1)
    sc = np.einsum('bhqd,bhkd->bhqk', q, k) / np.sqrt(HD) + np.asarray(mask).reshape(1, 1, S, S)
    sc = sc - sc.max(axis=-1, keepdims=True)
    e = np.exp(sc)
    a = e / e.sum(axis=-1, keepdims=True)
    o = np.einsum('bhqk,bhkd->bhqd', a, v)
    o = o.transpose(0, 2, 1, 3).reshape(B, S, H * HD)
    return (o @ Wo).astype(np.float32)


def kernel(x, Wq, Wk, Wv, Wo, qg, kg, cos, sin, mask, **_unused):
    x = np.asarray(x, dtype=np.float32)
    Wq, Wk, Wv, Wo = (np.asarray(a, dtype=np.float32) for a in (Wq, Wk, Wv, Wo))
    qg, kg = np.asarray(qg, np.float32), np.asarray(kg, np.float32)
    cos, sin = np.asarray(cos, np.float32), np.asarray(sin, np.float32)
    if not _causal_ok(mask):
        return _reference_fallback(x, Wq, Wk, Wv, Wo, qg, kg, cos, sin, mask)

    from concourse.bass_utils import run_bass_kernel_spmd

    if "nc" not in _CACHED:
        _CACHED["nc"] = _build_program()
    nc = _CACHED["nc"]

    cosT = np.ascontiguousarray(cos.T)  # [HD, S]
    sinT = np.ascontiguousarray(sin.T)

    # rope via halves: out[:64] = x[:64]*cos[:64] + x[64:]*sin_tbl[:64]
    #                  out[64:] = x[64:]*cos[64:] + x[:64]*sin_tbl[64:]
    # reference: rot(x)[:64] = -x[64:], rot(x)[64:] = x[:64]; gains fold in.
    def tables(g):
        ct = cosT * g[:, None]
        st = np.empty_like(sinT)
        st[:64] = -sinT[:64] * g[64:, None]
        st[64:] = sinT[64:] * g[:64, None]
        return ct.astype(BF), st.astype(BF)

    cq, sq = tables(qg)
    ck, sk = tables(kg)

    rsw = np.zeros((P, P), dtype=np.float32)
    for i in range(P):
        rsw[i, (i + 64) % P] = 1.0
    rsw = rsw.astype(BF)

    cols = np.arange(512)[None, :]
    rows = np.arange(P)[:, None]
    m4 = np.stack([(cols - P * a >= rows) for a in range(4)]).astype(BF)

    xT = [np.ascontiguousarray(x[b].T).astype(BF) for b in range(B)]

    in_maps = []
    for core in range(8):
        b, g = divmod(core, KV)
        in_maps.append({
            "xT": xT[b],
            "wq": np.ascontiguousarray(Wq[:, g * GQ * HD:(g + 1) * GQ * HD]).astype(BF),
            "wk": np.ascontiguousarray(Wk[:, g * HD:(g + 1) * HD]).astype(BF),
            "wv": np.ascontiguousarray(Wv[:, g * HD:(g + 1) * HD]).astype(BF),
            "wo": np.ascontiguousarray(Wo[g * GQ * HD:(g + 1) * GQ * HD, :]).astype(BF),
            "cosq": cq, "sinq": sq, "cosk": ck, "sink": sk,
            "m4": m4, "rsw": rsw,
        })

    res = run_bass_kernel_spmd(nc, in_maps, list(range(8)))
    out = np.zeros((B, S, DIM), dtype=np.float32)
    for core in range(8):
        out[core // KV] += res.results[core]["po"]
    return out



# revision 7
# speedup vs baseline: 1.2032x; 1.2032x over previous
"""GroupedQueryAttention Trainium2 kernel (8 NeuronCores).

Sharding: (batch b in 0..1) x (kv-head group g in 0..3) -> core 4*b+g.
Each core computes, for its batch, the 4 query heads (4g..4g+3) that share
kv head g, plus the partial output projection through the matching 512-row
slice of Wo.  The host sums the 4 partials per batch.

On-device dataflow is fully "transposed": activations live as [feature,
token] so every matmul contraction sits on the partition axis, and the
softmax probabilities come out directly in the layout the P@V matmul
needs.  The kernel is pipelined at 512-token chunk granularity:
projection (P1), rmsnorm+rope (P2), attention (P3) and the output
projection (P5) for successive chunks are interleaved so the PE never
idles long enough for the HAM clock gate to re-throttle.  All DRAM
operands are pre-packed on the host so every DMA moves long contiguous
runs per partition.  Softmax denominators are accumulated in f32 PSUM by
ones-matmuls over DVE-folded probability pairs; 1/den is computed as
exp(-ln(den)) on the scalar engine.
"""

import numpy as np
import ml_dtypes

DIM, H, KV, S, B = 2048, 16, 4, 2048, 2
HD = DIM // H          # 128
GQ = H // KV           # 4 query heads per kv head
P = 128                # partitions
NK = DIM // P          # 16 contraction tiles
CW = 512               # chunk width (tokens)
NCH = S // CW          # 4 sequence chunks
EPS = 1e-6
BF = ml_dtypes.bfloat16

_CACHED = {}


def _build_program():
    import concourse.bass as bass
    import concourse.tile as tile
    from concourse import bacc
    from concourse import mybir
    from concourse.masks import make_identity

    f32 = mybir.dt.float32
    bf16 = mybir.dt.bfloat16
    AF = mybir.ActivationFunctionType

    nc = bacc.Bacc()
    xt_d = [nc.declare_dram_parameter(f"xt{c}", [P, NK * CW], bf16, isOutput=False)
            for c in range(NCH)]
    wq = nc.declare_dram_parameter("wq", [P, NK * GQ * HD], bf16, isOutput=False)
    wk = nc.declare_dram_parameter("wk", [P, NK * HD], bf16, isOutput=False)
    wv = nc.declare_dram_parameter("wv", [P, NK * HD], bf16, isOutput=False)
    wo = nc.declare_dram_parameter("wo", [P, GQ * DIM], bf16, isOutput=False)
    cosq = nc.declare_dram_parameter("cosq", [HD, S], bf16, isOutput=False)
    sinq = nc.declare_dram_parameter("sinq", [HD, S], bf16, isOutput=False)
    cosk = nc.declare_dram_parameter("cosk", [HD, S], bf16, isOutput=False)
    sink = nc.declare_dram_parameter("sink", [HD, S], bf16, isOutput=False)
    m4 = nc.declare_dram_parameter("m4", [P, 4 * CW], bf16, isOutput=False)
    rsw = nc.declare_dram_parameter("rsw", [P, P], bf16, isOutput=False)
    po = nc.declare_dram_parameter("po", [S, DIM], bf16, isOutput=True)

    inv_sqrt_hd = 1.0 / float(np.sqrt(HD))

    with tile.TileContext(nc) as tc:
      with tc.tile_pool(name="const", bufs=1) as const, \
           tc.tile_pool(name="wts", bufs=1) as wts, \
           tc.tile_pool(name="hat", bufs=1) as hat, \
           tc.tile_pool(name="xtp", bufs=2) as xtp, \
           tc.tile_pool(name="q32p", bufs=8) as q32p, \
           tc.tile_pool(name="wk2", bufs=2) as wk2, \
           tc.tile_pool(name="ptp", bufs=10) as ptp, \
           tc.tile_pool(name="prt", bufs=6) as prt, \
           tc.tile_pool(name="rowp", bufs=2) as rowp, \
           tc.tile_pool(name="ps512", bufs=3, space="PSUM") as ps512, \
           tc.tile_pool(name="vtp", bufs=1, space="PSUM") as vtp, \
           tc.tile_pool(name="scp", bufs=2, space="PSUM") as scp:

        # ---- constants ----
        ones_sb = const.tile([P, P], bf16)
        nc.vector.memset(ones_sb, 1.0)
        ident = const.tile([P, P], bf16)
        make_identity(nc, ident)
        epsb = const.tile([P, 1], f32)
        nc.vector.memset(epsb, EPS)
        rsw_sb = const.tile([P, P], bf16)
        nc.sync.dma_start(out=rsw_sb, in_=rsw[:, :])

        # ---- persistent SBUF tensors ----
        wq_sb = wts.tile([P, NK, GQ * HD], bf16)
        wk_sb = wts.tile([P, NK, HD], bf16)
        wv_sb = wts.tile([P, NK, HD], bf16)
        wo_sb = wts.tile([P, GQ, DIM], bf16)
        m4_sb = wts.tile([P, 4, CW], bf16)
        cs_sb = {nm: wts.tile([P, S], bf16, name=f"cs_{nm}")
                 for nm in ("cosq", "sinq", "cosk", "sink")}

        khat = hat.tile([P, S], bf16, name="khat")
        qhat = [hat.tile([P, S], bf16, name=f"qhat{h}") for h in range(GQ)]
        v_nat = hat.tile([P, NK, HD], bf16, name="vnat")
        onorm = [hat.tile([P, S], bf16, name=f"onorm{h}") for h in range(GQ)]

        # ---- PE warm-up during the initial DMA wait (HAM un-throttle) ----
        wm0 = ps512.tile([P, CW], f32, tag="ps", name="warm0")
        wm1 = ps512.tile([P, CW], f32, tag="ps", name="warm1")
        for w in range(48):
            dst = wm0 if w % 2 == 0 else wm1
            nc.tensor.matmul(dst[:, 0:P], ident, ones_sb, start=True, stop=True)

        # ---- input DMAs, need-order ----
        nc.sync.dma_start(out=wk_sb, in_=wk.ap().rearrange("p (j n) -> p j n", j=NK))
        xt_sb = [None] * NCH
        xt_sb[0] = xtp.tile([P, NK, CW], bf16, tag="xt", name="xt0")
        nc.sync.dma_start(out=xt_sb[0], in_=xt_d[0].ap().rearrange("p (j t) -> p j t", j=NK))
        nc.sync.dma_start(out=wq_sb, in_=wq.ap().rearrange("p (j n) -> p j n", j=NK))
        nc.sync.dma_start(out=wv_sb, in_=wv.ap().rearrange("p (j n) -> p j n", j=NK))
        xt_sb[1] = xtp.tile([P, NK, CW], bf16, tag="xt", name="xt1")
        nc.sync.dma_start(out=xt_sb[1], in_=xt_d[1].ap().rearrange("p (j t) -> p j t", j=NK))
        for nm, t in (("cosk", cosk), ("sink", sink), ("cosq", cosq), ("sinq", sinq)):
            nc.sync.dma_start(out=cs_sb[nm], in_=t[:, :])
        nc.sync.dma_start(out=m4_sb, in_=m4.ap().rearrange("p (a n) -> p a n", a=4))
        nc.sync.dma_start(out=wo_sb, in_=wo.ap().rearrange("p (h n) -> p h n", h=GQ))

        def p1_chunk(c):
            """Projections for chunk c -> q32 tiles (bf16 SBUF), plus v
            transpose into v_nat.  Returns dict slot->sbuf tile.  k and v
            come first; the 4 v transposes are interleaved between the q
            projection slots so the single vt PSUM buffer never stalls PE."""
            out = {}
            for slot in (4, 5, 0, 1, 2, 3):
                ps = ps512.tile([P, CW], f32, tag="ps", name=f"proj_{c}_{slot}")
                for j in range(NK):
                    if slot < 4:
                        lhs = wq_sb[:, j, slot * HD:(slot + 1) * HD]
                    elif slot == 4:
                        lhs = wk_sb[:, j, :]
                    else:
                        lhs = wv_sb[:, j, :]
                    nc.tensor.matmul(ps, lhs, xt_sb[c][:, j, :],
                                     start=(j == 0), stop=(j == NK - 1))
                sb = q32p.tile([P, CW], bf16, tag="q32", name=f"q32_{c}_{slot}")
                nc.vector.tensor_copy(sb, ps)
                out[slot] = sb
                if slot < 4:
                    # v_nat[:, 4c+slot, :] = (v chunk block `slot`).T
                    tp = vtp.tile([P, HD], bf16, tag="vt", name=f"vt_{c}_{slot}")
                    nc.tensor.transpose(tp, out[5][:, slot * HD:(slot + 1) * HD], ident)
                    nc.vector.tensor_copy(v_nat[:, 4 * c + slot, :], tp)
            if c + 2 < NCH:
                xt_sb[c + 2] = xtp.tile([P, NK, CW], bf16, tag="xt", name=f"xt{c+2}")
                nc.sync.dma_start(
                    out=xt_sb[c + 2],
                    in_=xt_d[c + 2].ap().rearrange("p (j t) -> p j t", j=NK))
            return out

        def p2_chunk(c, q32):
            """rmsnorm (pre-gain) + rope for chunk c; writes khat/qhat."""
            sl = slice(c * CW, (c + 1) * CW)
            for t in (4, 0, 1, 2, 3):
                src = q32[t]
                dst = khat if t == 4 else qhat[t]
                cosT = cs_sb["cosk" if t == 4 else "cosq"]
                sinT = cs_sb["sink" if t == 4 else "sinq"]
                sqb = wk2.tile([P, CW], bf16, tag="sqb", name=f"sqb_{c}_{t}")
                nc.scalar.activation(sqb, src, AF.Square)
                ssq = ps512.tile([P, CW], f32, tag="ps", name=f"ssq_{c}_{t}")
                nc.tensor.matmul(ssq, ones_sb, sqb, start=True, stop=True)
                rot = ps512.tile([P, CW], f32, tag="ps", name=f"rot_{c}_{t}")
                nc.tensor.matmul(rot, rsw_sb, src, start=True, stop=True)
                lnb = wk2.tile([P, CW], f32, tag="lnb", name=f"lnb_{c}_{t}")
                nc.scalar.activation(lnb, ssq, AF.Ln, bias=epsb, scale=1.0 / HD)
                rsb = wk2.tile([P, CW], bf16, tag="rsb", name=f"rsb_{c}_{t}")
                nc.scalar.activation(rsb, lnb, AF.Exp, scale=-0.5)
                t1 = wk2.tile([P, CW], bf16, tag="t1", name=f"t1_{c}_{t}")
                nc.vector.tensor_mul(t1, src, cosT[:, sl])
                t2 = wk2.tile([P, CW], bf16, tag="t2", name=f"t2_{c}_{t}")
                nc.vector.tensor_mul(t2, rot, sinT[:, sl])
                t3 = wk2.tile([P, CW], bf16, tag="t3", name=f"t3_{c}_{t}")
                nc.vector.tensor_add(t3, t1, t2)
                nc.vector.tensor_mul(dst[:, sl], t3, rsb)

        def p3_chunk(c):
            """Causal attention for query chunk c, all 4 heads.  One head at
            a time: scores (PE) -> exp (ACT) -> mask (DVE) -> P@V + den
            accumulation (PE) -> 1/den (ACT) -> normalize (DVE)."""
            sl = slice(c * CW, (c + 1) * CW)
            nj = 4 * c + 4
            npr = nj // 2
            for h in range(GQ):
                ot = ps512.tile([P, CW], f32, tag="ps", name=f"ot_{c}_{h}")
                den = ps512.tile([P, CW], f32, tag="ps", name=f"den_{c}_{h}")

                def scores(pr):
                    """scores pair -> exp -> mask -> fold; returns (pt, part)."""
                    sc = scp.tile([P, 2 * CW], f32, tag="sc", name=f"sc_{c}_{h}_{pr}")
                    for u in range(2):
                        j = 2 * pr + u
                        nc.tensor.matmul(sc[:, u * CW:(u + 1) * CW],
                                         khat[:, j * P:(j + 1) * P],
                                         qhat[h][:, sl], start=True, stop=True)
                    pt = ptp.tile([P, 2 * CW], bf16, tag="pt", name=f"pt_{c}_{h}_{pr}")
                    nc.scalar.activation(pt, sc, AF.Exp, scale=inv_sqrt_hd)
                    for u in range(2):
                        j = 2 * pr + u
                        if j >= 4 * c:
                            usl = slice(u * CW, (u + 1) * CW)
                            nc.vector.tensor_mul(pt[:, usl], pt[:, usl],
                                                 m4_sb[:, j - 4 * c, :])
                    part = prt.tile([P, CW], bf16, tag="part", name=f"prt_{c}_{h}_{pr}")
                    nc.vector.tensor_add(part, pt[:, 0:CW], pt[:, CW:2 * CW])
                    return pt, part

                # lag-1 software pipeline: scores(pr+1) is issued before
                # P@V(pr) so the PE streams while ACT exps the previous pair
                cur = scores(0)
                for pr in range(npr):
                    nxt = scores(pr + 1) if pr + 1 < npr else None
                    pt, part = cur
                    for u in range(2):
                        j = 2 * pr + u
                        nc.tensor.matmul(ot, v_nat[:, j, :],
                                         pt[:, u * CW:(u + 1) * CW],
                                         start=(j == 0), stop=(j == nj - 1))
                    nc.tensor.matmul(den, ones_sb, part,
                                     start=(pr == 0), stop=(pr == npr - 1))
                    cur = nxt
                dln = wk2.tile([P, CW], f32, tag="dln", name=f"dln_{c}_{h}")
                nc.scalar.activation(dln, den, AF.Ln)
                drec = wk2.tile([P, CW], f32, tag="drec", name=f"drec_{c}_{h}")
                nc.scalar.activation(drec, dln, AF.Exp, scale=-1.0)
                nc.vector.tensor_mul(onorm[h][:, sl], ot, drec)

        def p5_chunk(c):
            """Partial output projection for chunk c's 4 token tiles.  Two
            n-columns at a time so only 2 PSUM accumulators are live."""
            for il in range(CW // P):
                i = 4 * c + il
                isl = slice(i * P, (i + 1) * P)
                row = rowp.tile([P, DIM], bf16, tag="row", name=f"row_{i}")
                for half in range(2):
                    pos = [ps512.tile([P, CW], f32, tag="ps", name=f"po_{i}_{half}_{n}")
                           for n in range(2)]
                    for h in range(GQ):
                        for n in range(2):
                            nn = 2 * half + n
                            nc.tensor.matmul(pos[n], onorm[h][:, isl],
                                             wo_sb[:, h, nn * CW:(nn + 1) * CW],
                                             start=(h == 0), stop=(h == GQ - 1))
                    for n in range(2):
                        nn = 2 * half + n
                        if half == 0:
                            nc.scalar.copy(row[:, nn * CW:(nn + 1) * CW], pos[n])
                        else:
                            nc.vector.tensor_copy(row[:, nn * CW:(nn + 1) * CW], pos[n])
                nc.sync.dma_start(out=po[isl, :], in_=row)

        q32 = p1_chunk(0)
        p2_chunk(0, q32)
        for c in range(NCH):
            p3_chunk(c)
            if c + 1 < NCH:
                q32 = p1_chunk(c + 1)
                p2_chunk(c + 1, q32)
            p5_chunk(c)
    nc.compile()
    return nc


def _causal_ok(mask):
    m = np.asarray(mask).reshape(S, S)
    tri = np.tril(np.ones((S, S), dtype=bool))
    return bool(np.all(m[tri] == 0.0) and np.all(m[~tri] <= -1e8))


def _reference_fallback(x, Wq, Wk, Wv, Wo, qg, kg, cos, sin, mask):
    x64 = np.asarray(x, dtype=np.float32)
    q = (x64 @ Wq).reshape(B, S, H, HD).transpose(0, 2, 1, 3)
    k = (x64 @ Wk).reshape(B, S, KV, HD).transpose(0, 2, 1, 3)
    v = (x64 @ Wv).reshape(B, S, KV, HD).transpose(0, 2, 1, 3)

    def rms(t, g):
        r = np.sqrt(np.mean(t * t, axis=-1, keepdims=True) + EPS)
        return g * (t / r)

    q, k = rms(q, qg), rms(k, kg)

    def rot(t):
        return np.concatenate([-t[..., HD // 2:], t[..., :HD // 2]], axis=-1)

    c = cos[None, None, :, :]
    s = sin[None, None, :, :]
    q = q * c + rot(q) * s
    k = k * c + rot(k) * s
    k = np.repeat(k, GQ, axis=1)
    v = np.repeat(v, GQ, axis=1)
    sc = np.einsum('bhqd,bhkd->bhqk', q, k) / np.sqrt(HD) + np.asarray(mask).reshape(1, 1, S, S)
    sc = sc - sc.max(axis=-1, keepdims=True)
    e = np.exp(sc)
    a = e / e.sum(axis=-1, keepdims=True)
    o = np.einsum('bhqk,bhkd->bhqd', a, v)
    o = o.transpose(0, 2, 1, 3).reshape(B, S, H * HD)
    return (o @ Wo).astype(np.float32)


def kernel(x, Wq, Wk, Wv, Wo, qg, kg, cos, sin, mask, **_unused):
    x = np.asarray(x, dtype=np.float32)
    Wq, Wk, Wv, Wo = (np.asarray(a, dtype=np.float32) for a in (Wq, Wk, Wv, Wo))
    qg, kg = np.asarray(qg, np.float32), np.asarray(kg, np.float32)
    cos, sin = np.asarray(cos, np.float32), np.asarray(sin, np.float32)
    if not _causal_ok(mask):
        return _reference_fallback(x, Wq, Wk, Wv, Wo, qg, kg, cos, sin, mask)

    from concourse.bass_utils import run_bass_kernel_spmd

    if "nc" not in _CACHED:
        _CACHED["nc"] = _build_program()
    nc = _CACHED["nc"]

    cosT = np.ascontiguousarray(cos.T)  # [HD, S]
    sinT = np.ascontiguousarray(sin.T)

    # rope via halves: out[:64] = x[:64]*cos[:64] + x[64:]*sin_tbl[:64]
    #                  out[64:] = x[64:]*cos[64:] + x[:64]*sin_tbl[64:]
    # reference: rot(x)[:64] = -x[64:], rot(x)[64:] = x[:64]; gains fold in.
    def tables(g):
        ct = cosT * g[:, None]
        st = np.empty_like(sinT)
        st[:64] = -sinT[:64] * g[64:, None]
        st[64:] = sinT[64:] * g[:64, None]
        return ct.astype(BF), st.astype(BF)

    cq, sq = tables(qg)
    ck, sk = tables(kg)

    rsw = np.zeros((P, P), dtype=np.float32)
    for i in range(P):
        rsw[i, (i + 64) % P] = 1.0
    rsw = rsw.astype(BF)

    cols = np.arange(CW)[None, :]
    rows = np.arange(P)[:, None]
    # m4 packed [P, 4*CW]: block a = 0/1 step mask for diagonal j-tile a
    m4 = np.concatenate([(cols - P * a >= rows) for a in range(4)],
                        axis=1).astype(BF)

    def pack_feat_major(w):        # [DIM, N] -> [P, NK*N] with (j p) rows
        n = w.shape[1]
        return np.ascontiguousarray(
            w.reshape(NK, P, n).transpose(1, 0, 2).reshape(P, NK * n)).astype(BF)

    in_maps = []
    for core in range(8):
        b, g = divmod(core, KV)
        xT = x[b].T                # [DIM, S]
        # xt{c}: [P, NK*CW] with xt[p, j*CW+t] = xT[j*P+p, c*CW+t]
        xp = xT.reshape(NK, P, NCH, CW).transpose(2, 1, 0, 3)
        im = {f"xt{c}": np.ascontiguousarray(xp[c].reshape(P, NK * CW)).astype(BF)
              for c in range(NCH)}
        woc = Wo[g * GQ * HD:(g + 1) * GQ * HD, :]   # [GQ*HD, DIM]
        im.update({
            "wq": pack_feat_major(Wq[:, g * GQ * HD:(g + 1) * GQ * HD]),
            "wk": pack_feat_major(Wk[:, g * HD:(g + 1) * HD]),
            "wv": pack_feat_major(Wv[:, g * HD:(g + 1) * HD]),
            "wo": np.ascontiguousarray(
                woc.reshape(GQ, P, DIM).transpose(1, 0, 2).reshape(P, GQ * DIM)).astype(BF),
            "cosq": cq, "sinq": sq, "cosk": ck, "sink": sk,
            "m4": m4, "rsw": rsw,
        })
        in_maps.append(im)

    res = run_bass_kernel_spmd(nc, in_maps, list(range(8)))
    out = np.zeros((B, S, DIM), dtype=np.float32)
    for core in range(8):
        out[core // KV] += np.asarray(res.results[core]["po"], dtype=np.float32)
    return out


# revision 11
# speedup vs baseline: 1.2616x; 1.0486x over previous
"""GroupedQueryAttention Trainium2 kernel (8 NeuronCores).

Sharding: (batch b in 0..1) x (kv-head group g in 0..3) -> core 4*b+g.
Each core computes, for its batch, the 4 query heads (4g..4g+3) that share
kv head g, plus the partial output projection through the matching 512-row
slice of Wo.  The host sums the 4 partials per batch.

On-device dataflow is fully "transposed": activations live as [feature,
token] so every matmul contraction sits on the partition axis, and the
softmax probabilities come out directly in the layout the P@V matmul
needs.  The kernel is pipelined at 512-token chunk granularity:
projection (P1), rmsnorm+rope (P2), attention (P3) and the output
projection (P5) for successive chunks are interleaved so the PE never
idles long enough for the HAM clock gate to re-throttle.  All DRAM
operands are pre-packed on the host so every DMA moves long contiguous
runs per partition.  Softmax denominators are accumulated in f32 PSUM by
ones-matmuls over DVE-folded probability pairs; 1/den is computed as
exp(-ln(den)) on the scalar engine.
"""

import numpy as np
import ml_dtypes

DIM, H, KV, S, B = 2048, 16, 4, 2048, 2
HD = DIM // H          # 128
GQ = H // KV           # 4 query heads per kv head
P = 128                # partitions
NK = DIM // P          # 16 contraction tiles
CW = 512               # chunk width (tokens)
NCH = S // CW          # 4 sequence chunks
EPS = 1e-6
BF = ml_dtypes.bfloat16

_CACHED = {}


def _build_program():
    import concourse.bass as bass
    import concourse.tile as tile
    from concourse import bacc
    from concourse import mybir
    from concourse.masks import make_identity

    # This kernel only uses Copy/Square/Ln/Exp, all of which live together
    # in the natural_log_exp_and_others table set.  The default per-activation
    # set choice bounces between exp_and_others and natural_log (57 table
    # loads, ~73us on ACT), so restrict those four functions to the one set
    # that holds them all; ids/ordering of the sets are preserved.
    AFT = mybir.ActivationFunctionType
    _orig_tables = bacc.get_activation_tables

    def _pinned_tables(arch):
        tabs = {k: set(v) for k, v in _orig_tables(arch).items()}
        mine = {AFT.Copy, AFT.Square, AFT.Ln, AFT.Exp}
        for name, fns in tabs.items():
            if name != "natural_log_exp_and_others":
                fns -= mine
        return tabs

    bacc.get_activation_tables = _pinned_tables

    f32 = mybir.dt.float32
    bf16 = mybir.dt.bfloat16
    AF = mybir.ActivationFunctionType

    nc = bacc.Bacc()
    xt_d = [nc.declare_dram_parameter(f"xt{c}", [P, NK * CW], bf16, isOutput=False)
            for c in range(NCH)]
    wq = nc.declare_dram_parameter("wq", [P, NK * GQ * HD], bf16, isOutput=False)
    wk = nc.declare_dram_parameter("wk", [P, NK * HD], bf16, isOutput=False)
    wv = nc.declare_dram_parameter("wv", [P, NK * HD], bf16, isOutput=False)
    wo = nc.declare_dram_parameter("wo", [P, GQ * DIM], bf16, isOutput=False)
    cosq = nc.declare_dram_parameter("cosq", [HD, S], bf16, isOutput=False)
    sinq = nc.declare_dram_parameter("sinq", [HD, S], bf16, isOutput=False)
    cosk = nc.declare_dram_parameter("cosk", [HD, S], bf16, isOutput=False)
    sink = nc.declare_dram_parameter("sink", [HD, S], bf16, isOutput=False)
    m4 = nc.declare_dram_parameter("m4", [P, 4 * CW], bf16, isOutput=False)
    rsw = nc.declare_dram_parameter("rsw", [P, P], bf16, isOutput=False)
    po = nc.declare_dram_parameter("po", [S, DIM], bf16, isOutput=True)

    inv_sqrt_hd = 1.0 / float(np.sqrt(HD))

    with tile.TileContext(nc) as tc:
      with tc.tile_pool(name="const", bufs=1) as const, \
           tc.tile_pool(name="wts", bufs=1) as wts, \
           tc.tile_pool(name="hat", bufs=1) as hat, \
           tc.tile_pool(name="xtp", bufs=2) as xtp, \
           tc.tile_pool(name="q32p", bufs=8) as q32p, \
           tc.tile_pool(name="wk2", bufs=2) as wk2, \
           tc.tile_pool(name="ptp", bufs=10) as ptp, \
           tc.tile_pool(name="prt", bufs=6) as prt, \
           tc.tile_pool(name="rowp", bufs=2) as rowp, \
           tc.tile_pool(name="ps512", bufs=3, space="PSUM") as ps512, \
           tc.tile_pool(name="vtp", bufs=1, space="PSUM") as vtp, \
           tc.tile_pool(name="scp", bufs=2, space="PSUM") as scp:

        # ---- constants ----
        ones_wide = const.tile([P, CW], bf16)
        nc.vector.memset(ones_wide, 1.0)
        ones_sb = ones_wide[:, 0:P]
        ident = const.tile([P, P], bf16)
        make_identity(nc, ident)
        epsb = const.tile([P, 1], f32)
        nc.vector.memset(epsb, EPS)
        rsw_sb = const.tile([P, P], bf16)

        # ---- persistent SBUF tensors ----
        wq_sb = wts.tile([P, NK, GQ * HD], bf16)
        wk_sb = wts.tile([P, NK, HD], bf16)
        wv_sb = wts.tile([P, NK, HD], bf16)
        wo_sb = wts.tile([P, GQ, DIM], bf16)
        m4_sb = wts.tile([P, 4, CW], bf16)
        cs_sb = {nm: wts.tile([P, S], bf16, name=f"cs_{nm}")
                 for nm in ("cosq", "sinq", "cosk", "sink")}

        khat = hat.tile([P, S], bf16, name="khat")
        qhat = [hat.tile([P, S], bf16, name=f"qhat{h}") for h in range(GQ)]
        v_nat = hat.tile([P, NK, HD], bf16, name="vnat")
        onorm = [hat.tile([P, S], bf16, name=f"onorm{h}") for h in range(GQ)]

        # ---- input DMAs, need-order (wk+xt0 first so P1 starts early) ----
        nc.sync.dma_start(out=wk_sb, in_=wk.ap().rearrange("p (j n) -> p j n", j=NK))
        xt_sb = [None] * NCH
        xt_sb[0] = xtp.tile([P, NK, CW], bf16, tag="xt", name="xt0")
        nc.sync.dma_start(out=xt_sb[0], in_=xt_d[0].ap().rearrange("p (j t) -> p j t", j=NK))
        nc.sync.dma_start(out=wv_sb, in_=wv.ap().rearrange("p (j n) -> p j n", j=NK))
        nc.sync.dma_start(out=wq_sb, in_=wq.ap().rearrange("p (j n) -> p j n", j=NK))
        nc.sync.dma_start(out=rsw_sb, in_=rsw[:, :])
        xt_sb[1] = xtp.tile([P, NK, CW], bf16, tag="xt", name="xt1")
        nc.sync.dma_start(out=xt_sb[1], in_=xt_d[1].ap().rearrange("p (j t) -> p j t", j=NK))
        for nm, t in (("cosk", cosk), ("sink", sink), ("cosq", cosq), ("sinq", sinq)):
            nc.sync.dma_start(out=cs_sb[nm], in_=t[:, :])
        nc.sync.dma_start(out=m4_sb, in_=m4.ap().rearrange("p (a n) -> p a n", a=4))
        nc.sync.dma_start(out=wo_sb, in_=wo.ap().rearrange("p (h n) -> p h n", h=GQ))

        # ---- PE warm-up during the initial DMA wait (HAM un-throttle) ----
        wm0 = ps512.tile([P, CW], f32, tag="ps", name="warm0")
        wm1 = ps512.tile([P, CW], f32, tag="ps", name="warm1")
        for w in range(12):
            nc.tensor.matmul(wm0 if w % 2 == 0 else wm1, ident,
                             ones_wide, start=True, stop=True)

        def p1_chunk(c):
            """Projections for chunk c -> q32 tiles (bf16 SBUF), plus v
            transpose into v_nat.  Returns dict slot->sbuf tile.  k and v
            come first; the 4 v transposes are interleaved between the q
            projection slots so the single vt PSUM buffer never stalls PE."""
            out = {}
            for slot in (4, 5, 0, 1, 2, 3):
                ps = ps512.tile([P, CW], f32, tag="ps", name=f"proj_{c}_{slot}")
                for j in range(NK):
                    if slot < 4:
                        lhs = wq_sb[:, j, slot * HD:(slot + 1) * HD]
                    elif slot == 4:
                        lhs = wk_sb[:, j, :]
                    else:
                        lhs = wv_sb[:, j, :]
                    nc.tensor.matmul(ps, lhs, xt_sb[c][:, j, :],
                                     start=(j == 0), stop=(j == NK - 1))
                sb = q32p.tile([P, CW], bf16, tag="q32", name=f"q32_{c}_{slot}")
                nc.vector.tensor_copy(sb, ps)
                out[slot] = sb
                if slot < 4:
                    # v_nat[:, 4c+slot, :] = (v chunk block `slot`).T
                    tp = vtp.tile([P, HD], bf16, tag="vt", name=f"vt_{c}_{slot}")
                    nc.tensor.transpose(tp, out[5][:, slot * HD:(slot + 1) * HD], ident)
                    nc.vector.tensor_copy(v_nat[:, 4 * c + slot, :], tp)
            if c + 2 < NCH:
                xt_sb[c + 2] = xtp.tile([P, NK, CW], bf16, tag="xt", name=f"xt{c+2}")
                nc.sync.dma_start(
                    out=xt_sb[c + 2],
                    in_=xt_d[c + 2].ap().rearrange("p (j t) -> p j t", j=NK))
            return out

        def p2_chunk(c, q32):
            """rmsnorm (pre-gain) + rope for chunk c; writes khat/qhat."""
            sl = slice(c * CW, (c + 1) * CW)
            for t in (4, 0, 1, 2, 3):
                src = q32[t]
                dst = khat if t == 4 else qhat[t]
                cosT = cs_sb["cosk" if t == 4 else "cosq"]
                sinT = cs_sb["sink" if t == 4 else "sinq"]
                sqb = wk2.tile([P, CW], bf16, tag="sqb", name=f"sqb_{c}_{t}")
                nc.scalar.activation(sqb, src, AF.Square)
                ssq = ps512.tile([P, CW], f32, tag="ps", name=f"ssq_{c}_{t}")
                nc.tensor.matmul(ssq, ones_sb, sqb, start=True, stop=True)
                rot = ps512.tile([P, CW], f32, tag="ps", name=f"rot_{c}_{t}")
                nc.tensor.matmul(rot, rsw_sb, src, start=True, stop=True)
                lnb = wk2.tile([P, CW], f32, tag="lnb", name=f"lnb_{c}_{t}")
                nc.scalar.activation(lnb, ssq, AF.Ln, bias=epsb, scale=1.0 / HD)
                rsb = wk2.tile([P, CW], bf16, tag="rsb", name=f"rsb_{c}_{t}")
                nc.scalar.activation(rsb, lnb, AF.Exp, scale=-0.5)
                t1 = wk2.tile([P, CW], bf16, tag="t1", name=f"t1_{c}_{t}")
                nc.vector.tensor_mul(t1, src, cosT[:, sl])
                t2 = wk2.tile([P, CW], bf16, tag="t2", name=f"t2_{c}_{t}")
                nc.vector.tensor_mul(t2, rot, sinT[:, sl])
                t3 = wk2.tile([P, CW], bf16, tag="t3", name=f"t3_{c}_{t}")
                nc.vector.tensor_add(t3, t1, t2)
                nc.vector.tensor_mul(dst[:, sl], t3, rsb)

        def p3_chunk(c):
            """Causal attention for query chunk c, all 4 heads.  One head at
            a time: scores (PE) -> exp (ACT) -> mask (DVE) -> P@V + den
            accumulation (PE) -> 1/den (ACT) -> normalize (DVE)."""
            sl = slice(c * CW, (c + 1) * CW)
            nj = 4 * c + 4
            npr = nj // 2
            for h in range(GQ):
                ot = ps512.tile([P, CW], f32, tag="ps", name=f"ot_{c}_{h}")
                den = ps512.tile([P, CW], f32, tag="ps", name=f"den_{c}_{h}")

                def scores(pr):
                    """scores pair -> exp -> mask -> fold; returns (pt, part)."""
                    sc = scp.tile([P, 2 * CW], f32, tag="sc", name=f"sc_{c}_{h}_{pr}")
                    for u in range(2):
                        j = 2 * pr + u
                        nc.tensor.matmul(sc[:, u * CW:(u + 1) * CW],
                                         khat[:, j * P:(j + 1) * P],
                                         qhat[h][:, sl], start=True, stop=True)
                    pt = ptp.tile([P, 2 * CW], bf16, tag="pt", name=f"pt_{c}_{h}_{pr}")
                    nc.scalar.activation(pt, sc, AF.Exp, scale=inv_sqrt_hd)
                    for u in range(2):
                        j = 2 * pr + u
                        if j >= 4 * c:
                            usl = slice(u * CW, (u + 1) * CW)
                            nc.vector.tensor_mul(pt[:, usl], pt[:, usl],
                                                 m4_sb[:, j - 4 * c, :])
                    part = prt.tile([P, CW], bf16, tag="part", name=f"prt_{c}_{h}_{pr}")
                    nc.vector.tensor_add(part, pt[:, 0:CW], pt[:, CW:2 * CW])
                    return pt, part

                # lag-1 software pipeline: scores(pr+1) is issued before
                # P@V(pr) so the PE streams while ACT exps the previous pair
                cur = scores(0)
                for pr in range(npr):
                    nxt = scores(pr + 1) if pr + 1 < npr else None
                    pt, part = cur
                    for u in range(2):
                        j = 2 * pr + u
                        nc.tensor.matmul(ot, v_nat[:, j, :],
                                         pt[:, u * CW:(u + 1) * CW],
                                         start=(j == 0), stop=(j == nj - 1))
                    nc.tensor.matmul(den, ones_sb, part,
                                     start=(pr == 0), stop=(pr == npr - 1))
                    cur = nxt
                dln = wk2.tile([P, CW], f32, tag="dln", name=f"dln_{c}_{h}")
                nc.scalar.activation(dln, den, AF.Ln)
                drec = wk2.tile([P, CW], f32, tag="drec", name=f"drec_{c}_{h}")
                nc.scalar.activation(drec, dln, AF.Exp, scale=-1.0)
                nc.vector.tensor_mul(onorm[h][:, sl], ot, drec)

        def p5_chunk(c):
            """Partial output projection for chunk c's 4 token tiles.  Two
            n-columns at a time so only 2 PSUM accumulators are live."""
            for il in range(CW // P):
                i = 4 * c + il
                isl = slice(i * P, (i + 1) * P)
                row = rowp.tile([P, DIM], bf16, tag="row", name=f"row_{i}")
                for half in range(2):
                    pos = [ps512.tile([P, CW], f32, tag="ps", name=f"po_{i}_{half}_{n}")
                           for n in range(2)]
                    for h in range(GQ):
                        for n in range(2):
                            nn = 2 * half + n
                            nc.tensor.matmul(pos[n], onorm[h][:, isl],
                                             wo_sb[:, h, nn * CW:(nn + 1) * CW],
                                             start=(h == 0), stop=(h == GQ - 1))
                    for n in range(2):
                        nn = 2 * half + n
                        if half == 0:
                            nc.scalar.copy(row[:, nn * CW:(nn + 1) * CW], pos[n])
                        else:
                            nc.vector.tensor_copy(row[:, nn * CW:(nn + 1) * CW], pos[n])
                nc.sync.dma_start(out=po[isl, :], in_=row)

        q32 = p1_chunk(0)
        p2_chunk(0, q32)
        for c in range(NCH):
            p3_chunk(c)
            if c + 1 < NCH:
                q32 = p1_chunk(c + 1)
                p2_chunk(c + 1, q32)
            p5_chunk(c)
    nc.compile()
    bacc.get_activation_tables = _orig_tables
    return nc


def _causal_ok(mask):
    m = np.asarray(mask).reshape(S, S)
    tri = np.tril(np.ones((S, S), dtype=bool))
    return bool(np.all(m[tri] == 0.0) and np.all(m[~tri] <= -1e8))


def _reference_fallback(x, Wq, Wk, Wv, Wo, qg, kg, cos, sin, mask):
    x64 = np.asarray(x, dtype=np.float32)
    q = (x64 @ Wq).reshape(B, S, H, HD).transpose(0, 2, 1, 3)
    k = (x64 @ Wk).reshape(B, S, KV, HD).transpose(0, 2, 1, 3)
    v = (x64 @ Wv).reshape(B, S, KV, HD).transpose(0, 2, 1, 3)

    def rms(t, g):
        r = np.sqrt(np.mean(t * t, axis=-1, keepdims=True) + EPS)
        return g * (t / r)

    q, k = rms(q, qg), rms(k, kg)

    def rot(t):
        return np.concatenate([-t[..., HD // 2:], t[..., :HD // 2]], axis=-1)

    c = cos[None, None, :, :]
    s = sin[None, None, :, :]
    q = q * c + rot(q) * s
    k = k * c + rot(k) * s
    k = np.repeat(k, GQ, axis=1)
    v = np.repeat(v, GQ, axis=1)
    sc = np.einsum('bhqd,bhkd->bhqk', q, k) / np.sqrt(HD) + np.asarray(mask).reshape(1, 1, S, S)
    sc = sc - sc.max(axis=-1, keepdims=True)
    e = np.exp(sc)
    a = e / e.sum(axis=-1, keepdims=True)
    o = np.einsum('bhqk,bhkd->bhqd', a, v)
    o = o.transpose(0, 2, 1, 3).reshape(B, S, H * HD)
    return (o @ Wo).astype(np.float32)


def kernel(x, Wq, Wk, Wv, Wo, qg, kg, cos, sin, mask, **_unused):
    x = np.asarray(x, dtype=np.float32)
    Wq, Wk, Wv, Wo = (np.asarray(a, dtype=np.float32) for a in (Wq, Wk, Wv, Wo))
    qg, kg = np.asarray(qg, np.float32), np.asarray(kg, np.float32)
    cos, sin = np.asarray(cos, np.float32), np.asarray(sin, np.float32)
    if not _causal_ok(mask):
        return _reference_fallback(x, Wq, Wk, Wv, Wo, qg, kg, cos, sin, mask)

    from concourse.bass_utils import run_bass_kernel_spmd

    if "nc" not in _CACHED:
        _CACHED["nc"] = _build_program()
    nc = _CACHED["nc"]

    cosT = np.ascontiguousarray(cos.T)  # [HD, S]
    sinT = np.ascontiguousarray(sin.T)

    # rope via halves: out[:64] = x[:64]*cos[:64] + x[64:]*sin_tbl[:64]
    #                  out[64:] = x[64:]*cos[64:] + x[:64]*sin_tbl[64:]
    # reference: rot(x)[:64] = -x[64:], rot(x)[64:] = x[:64]; gains fold in.
    def tables(g):
        ct = cosT * g[:, None]
        st = np.empty_like(sinT)
        st[:64] = -sinT[:64] * g[64:, None]
        st[64:] = sinT[64:] * g[:64, None]
        return ct.astype(BF), st.astype(BF)

    cq, sq = tables(qg)
    ck, sk = tables(kg)

    rsw = np.zeros((P, P), dtype=np.float32)
    for i in range(P):
        rsw[i, (i + 64) % P] = 1.0
    rsw = rsw.astype(BF)

    cols = np.arange(CW)[None, :]
    rows = np.arange(P)[:, None]
    # m4 packed [P, 4*CW]: block a = 0/1 step mask for diagonal j-tile a
    m4 = np.concatenate([(cols - P * a >= rows) for a in range(4)],
                        axis=1).astype(BF)

    def pack_feat_major(w):        # [DIM, N] -> [P, NK*N] with (j p) rows
        n = w.shape[1]
        return np.ascontiguousarray(
            w.reshape(NK, P, n).transpose(1, 0, 2).reshape(P, NK * n)).astype(BF)

    in_maps = []
    for core in range(8):
        b, g = divmod(core, KV)
        xT = x[b].T                # [DIM, S]
        # xt{c}: [P, NK*CW] with xt[p, j*CW+t] = xT[j*P+p, c*CW+t]
        xp = xT.reshape(NK, P, NCH, CW).transpose(2, 1, 0, 3)
        im = {f"xt{c}": np.ascontiguousarray(xp[c].reshape(P, NK * CW)).astype(BF)
              for c in range(NCH)}
        woc = Wo[g * GQ * HD:(g + 1) * GQ * HD, :]   # [GQ*HD, DIM]
        im.update({
            "wq": pack_feat_major(Wq[:, g * GQ * HD:(g + 1) * GQ * HD]),
            "wk": pack_feat_major(Wk[:, g * HD:(g + 1) * HD]),
            "wv": pack_feat_major(Wv[:, g * HD:(g + 1) * HD]),
            "wo": np.ascontiguousarray(
                woc.reshape(GQ, P, DIM).transpose(1, 0, 2).reshape(P, GQ * DIM)).astype(BF),
            "cosq": cq, "sinq": sq, "cosk": ck, "sink": sk,
            "m4": m4, "rsw": rsw,
        })
        in_maps.append(im)

    res = run_bass_kernel_spmd(nc, in_maps, list(range(8)))
    out = np.zeros((B, S, DIM), dtype=np.float32)
    for core in range(8):
        out[core // KV] += np.asarray(res.results[core]["po"], dtype=np.float32)
    return out
